# revision 23
# baseline (speedup 1.0000x reference)
"""Chamfer loss kernel for Trainium2, SPMD over 8 NeuronCores.

Problem: rec (4, 8192, 3), data (4, 8192, 3) float32 ->
scalar = mean_b max( mean_i min_j d[b,i,j], mean_j min_i d[b,i,j] )
with d = squared euclidean distance, clamped at 0.

Strategy: 8 cores = 4 batches x 2 directions. Core c handles batch c//2,
direction c%2 (direction 0: rows=rec, cols=data; direction 1: swapped).
Each core computes row-mins of e[i,j] = |q_j|^2 - 2 p_i . q_j over all
8192 columns for its 8192 rows; the host adds |p_i|^2, clamps, and does
the tiny means/max/mean. No cross-core communication needed.

The dot products run on the TensorEngine as a K=14 augmented matmul in
bf16 with hi/lo splitting (near-fp32 precision, 1 cycle/column): the
term |q|^2 - 2 p.q is one augmented inner product over
[-2ph, -2pl, -2ph, -2pl, 1, 1] x [qh, qh, ql, ql, sqh, sql]. Row mins
run on the VectorEngine as multi-bank tensor_reduce straight out of
PSUM (4 banks / 2048 elems per instruction, two 4-bank groups
ping-ponging against the matmul fills). On this hardware PSUM can only
be read by the VectorEngine (ScalarE reads crash the device, DMA and
GpSimd have no port, and only one DVE operand may live in PSUM), so
the 1 fp32/lane/cycle PSUM port is the roofline; this kernel runs at
~95% of it.
"""

import numpy as np
import ml_dtypes

import concourse.bacc as bacc
import concourse.tile as tile
from concourse import mybir
from concourse.bass_utils import run_bass_kernel_spmd

NPTS = 8192          # points per batch on each side
NB = 4               # batches
KT = 14              # augmented contraction dim
RT = NPTS // 128     # 64 row tiles of 128 points
CT = NPTS // 512     # 16 col tiles of 512 points

_BF16 = ml_dtypes.bfloat16

_prog_cache = {}


def _build_program():
    key = "prog"
    if key in _prog_cache:
        return _prog_cache[key]
    nc = bacc.Bacc("TRN2", target_bir_lowering=False, debug=False, num_devices=8)
    lhsT_d = nc.dram_tensor("lhsT", [KT, NPTS], mybir.dt.bfloat16,
                            kind="ExternalInput").ap()
    rhsT_d = nc.dram_tensor("rhsT", [KT, NPTS], mybir.dt.bfloat16,
                            kind="ExternalInput").ap()
    out_d = nc.dram_tensor("out", [128, RT], mybir.dt.float32,
                           kind="ExternalOutput").ap()

    f32 = mybir.dt.float32
    mn = mybir.AluOpType.min

    with tile.TileContext(nc) as tc:
        with (
            tc.tile_pool(name="singles", bufs=1) as singles,
            tc.tile_pool(name="psum", bufs=1, space="PSUM") as psum_pool,
        ):
            lhs_sb = singles.tile([KT, NPTS], mybir.dt.bfloat16)
            rhs_sb = singles.tile([KT, NPTS], mybir.dt.bfloat16)
            # chunked loads, first-group data first (row tile 0 needs
            # lhs cols 0:128 and rhs cols 0:2048), spread over engine
            # queues so descriptor generation overlaps
            nc.gpsimd.dma_start(out=lhs_sb[:, 0:128], in_=lhsT_d[:, 0:128])
            nc.sync.dma_start(out=rhs_sb[:, 0:1024], in_=rhsT_d[:, 0:1024])
            nc.scalar.dma_start(out=rhs_sb[:, 1024:2048],
                                in_=rhsT_d[:, 1024:2048])
            nc.sync.dma_start(out=rhs_sb[:, 2048:8192],
                              in_=rhsT_d[:, 2048:8192])
            nc.sync.dma_start(out=lhs_sb[:, 128:8192], in_=lhsT_d[:, 128:8192])
            out_sb = singles.tile([128, RT], f32)
            # 7 partial-min slots per row tile: row 0 splits its first
            # group into four 1-bank reduces so the DVE starts right
            # after the first matmul; unused slots hold +inf from the
            # memset
            parts_all = singles.tile([128, RT, 7], f32)
            nc.gpsimd.memset(parts_all, 3.0e38)

            for r in range(RT):
                lhs_slice = lhs_sb[:, r * 128:(r + 1) * 128]
                # 4 groups of 4 col tiles each; two 4-bank PSUM tags
                # ping-pong so the matmul fills overlap the reduces.
                # Per-group partial mins collect into parts_all,
                # reduced once at the very end.
                for g in range(4):
                    ps = psum_pool.tile([128, 4, 512], f32,
                                        tag=f"psg{g % 2}")
                    for i in range(4):
                        c = g * 4 + i
                        nc.tensor.matmul(
                            ps[:, i, :], lhs_slice,
                            rhs_sb[:, c * 512:(c + 1) * 512],
                            start=True, stop=True)
                    if r == 0 and g == 0:
                        # split: start reducing after the first matmul
                        for i in range(4):
                            slot = (0, 4, 5, 6)[i]
                            nc.vector.tensor_reduce(
                                out=parts_all[:, r, slot:slot + 1],
                                in_=ps[:, i, :],
                                axis=mybir.AxisListType.X, op=mn)
                    else:
                        nc.vector.tensor_reduce(
                            out=parts_all[:, r, g:g + 1], in_=ps,
                            axis=mybir.AxisListType.XY, op=mn)

            nc.vector.tensor_reduce(
                out=out_sb, in_=parts_all,
                axis=mybir.AxisListType.X, op=mn)
            nc.sync.dma_start(out=out_d, in_=out_sb)

    nc.compile()
    _prog_cache[key] = nc
    return nc


def _split_bf16(x):
    h = x.astype(_BF16).astype(np.float32)
    l = (x - h).astype(_BF16).astype(np.float32)
    return h, l


def _prep_core(P, Q):
    """Augmented operands for row-mins of |q_j|^2 - 2 p_i . q_j."""
    ph, pl = _split_bf16(P)              # (NPTS, 3)
    qh, ql = _split_bf16(Q)
    sq = np.sum(Q.astype(np.float64) * Q.astype(np.float64),
                axis=1).astype(np.float32)
    sqh, sql = _split_bf16(sq)
    ones = np.ones((1, NPTS), np.float32)
    lhsT = np.concatenate([
        (-2.0 * ph).T, (-2.0 * pl).T, (-2.0 * ph).T, (-2.0 * pl).T,
        ones, ones,
    ], axis=0).astype(_BF16)             # (14, NPTS)
    rhsT = np.concatenate([
        qh.T, qh.T, ql.T, ql.T, sqh[None, :], sql[None, :],
    ], axis=0).astype(_BF16)             # (14, NPTS)
    sp = np.sum(P.astype(np.float64) * P.astype(np.float64),
                axis=1).astype(np.float32)
    return lhsT, rhsT, sp


def _run(rec, data, trace=False):
    rec = np.asarray(rec, dtype=np.float32)
    data = np.asarray(data, dtype=np.float32)
    assert rec.shape == (NB, NPTS, 3) and data.shape == (NB, NPTS, 3)

    in_maps = []
    sps = []
    for c in range(8):
        b, d = c // 2, c % 2
        P, Q = (rec[b], data[b]) if d == 0 else (data[b], rec[b])
        lhsT, rhsT, sp = _prep_core(P, Q)
        in_maps.append({"lhsT": lhsT, "rhsT": rhsT})
        sps.append(sp)

    nc = _build_program()
    res = run_bass_kernel_spmd(nc, in_maps, core_ids=list(range(8)),
                               trace=trace)

    means = []
    for c in range(8):
        arr = np.asarray(res.results[c]["out"])      # (128, RT)
        vec = arr.T.reshape(NPTS)                    # index r*128 + p
        dmin = np.maximum(vec + sps[c], 0.0)
        means.append(np.mean(dmin.astype(np.float64)))
    per_batch = [max(means[2 * b], means[2 * b + 1]) for b in range(NB)]
    result = np.asarray(np.mean(per_batch), dtype=np.float32)
    return result, res


def kernel(rec, data):
    return _run(rec, data, trace=False)[0]
